# revision 36
# baseline (speedup 1.0000x reference)
#!/usr/bin/env python3
"""Bass/Trainium2 kernel for nn_Attention_12747462934680.

Reference computation (B=64, L=2048, H=512):
    x = concat([hidden broadcast over L, encoder_outputs], -1)   # [B, L, 2H]
    energy = tanh(x @ W.T + b)                                   # [B, L, H]
    scores = energy @ v                                          # [B, L]
    attn = softmax(scores, axis=1)[:, None, :]                   # [B, 1, L]

Decomposition:
    pre[b,l,h] = (enc[b,l] @ W2.T)[h] + (hidden[b] @ W1.T)[h] + bias[h]
    with W1 = W[:, :H], W2 = W[:, H:].  The hidden term is per-(b,h), computed
    once; the big matmul is enc @ W2.T.

Sharding: data-parallel over B across 8 cores (8 batches/core).

Per-core device pipeline (SPMD, no collectives), data path in fp16
(~11-bit mantissa — same error class as the PE's tf32-like f32r mode, but
transposes run at 1 cyc/row and weight loads get FWL):
  - h1T[h, b] = W1T.T @ hiddenT + bias  (tiny matmul, ACT adds bias)
  - software-pipelined loop over 32 (l-chunk j, batch b) groups of 512
    tokens, j-major so each l-chunk completes for all 8 b before the next:
      SWDGE DMA enc[512, 512] natural, casting f32 -> fp16
      16x PE transpose [128,128] -> PSUM, DVE copy -> encT (k on partitions)
      preT[h, t] = W2T.T @ encT  (fp16 matmul, fp32 PSUM)
      energy = tanh(preT + h1T[:, b]) on ACT (PSUM -> SBUF, fp16)
      scores: vmat.T @ energy with M=36; vmat is one-hot so batch b lands on
      32-aligned row 32*(b//4)+b%4, and all 8 b of a chunk ACCUMULATE into
      one [36, 512] PSUM tile -> the chunk's scores assemble in PSUM with no
      partition-scatter DMAs (engines cannot shift partitions)
  - once a chunk completes: online-softmax update reads that PSUM directly:
    running max m, exp-chunk into expstore[36, L] (ACT, accum_out gives the
    chunk sum for free), running rescaled sum s = s*exp(m_old-m_new) + csum
  - tail is only the final rescale: attn = expstore * exp(m_j - m)/s with
    vectorized scale math, two output DMAs  (~7 us instead of 14)
"""
import sys
import numpy as np

sys.path.insert(0, "/opt/trn_rl_repo")

B, L, H = 64, 2048, 512
NCORES = 8
BPC = B // NCORES          # batches per core
T = BPC * L                # tokens per core = 16384
GT = 512                   # tokens per group
G = T // GT                # 32 groups
NJ = L // GT               # 4 l-chunks per batch
KT = H // 128              # 4 k-tiles
HT = H // 128              # 4 h-tiles

_compiled = None


def _build(variant="full"):
    from contextlib import ExitStack
    from concourse import bacc, mybir
    import concourse.tile as tile
    from concourse.bass import ts

    f32 = mybir.dt.float32
    fp16 = mybir.dt.float16
    DT = fp16
    ActF = mybir.ActivationFunctionType

    nc = bacc.Bacc("TRN2", target_bir_lowering=False, debug=False,
                   enable_asserts=True, num_devices=NCORES)

    enc_d = nc.dram_tensor("enc", [T, H], f32, kind="ExternalInput").ap()
    w2t_d = nc.dram_tensor("w2t", [H, H], f32, kind="ExternalInput").ap()
    w1t_d = nc.dram_tensor("w1t", [H, H], f32, kind="ExternalInput").ap()
    hidT_d = nc.dram_tensor("hidT", [H, BPC], f32, kind="ExternalInput").ap()
    bvec_d = nc.dram_tensor("bvec", [H], f32, kind="ExternalInput").ap()
    vmat_d = nc.dram_tensor("vmat", [HT, 128, 36, BPC], f32,
                            kind="ExternalInput").ap()
    ident_d = nc.dram_tensor("ident", [128, 128], fp16, kind="ExternalInput").ap()
    attn_d = nc.dram_tensor("attn", [BPC, L], f32, kind="ExternalOutput").ap()

    with tile.TileContext(nc) as tc:
        with ExitStack() as ctx:
            singles = ctx.enter_context(tc.tile_pool(name="singles", bufs=1))
            encp = ctx.enter_context(tc.tile_pool(name="encp", bufs=3))
            enctp = ctx.enter_context(tc.tile_pool(name="enctp", bufs=3))
            enrgp = ctx.enter_context(tc.tile_pool(name="enrgp", bufs=10))
            smp = ctx.enter_context(tc.tile_pool(name="smp", bufs=3))
            psT = ctx.enter_context(tc.tile_pool(name="psT", bufs=3, space="PSUM"))
            psP = ctx.enter_context(tc.tile_pool(name="psP", bufs=3, space="PSUM"))
            psS = ctx.enter_context(tc.tile_pool(name="psS", bufs=2, space="PSUM"))

            # ---- constants / params ----
            # params ride the HWDGE (sync) queue so SWDGE streams enc
            # immediately; DVE casts them to fp16.
            ident_r = singles.tile([128, 128], DT, tag="identr")
            nc.sync.dma_start(out=ident_r, in_=ident_d)
            w2t_f = singles.tile([128, KT, H], f32, tag="w2tf")
            nc.sync.dma_start(out=w2t_f,
                              in_=w2t_d.rearrange("(kt p) h -> p kt h", p=128))
            w2t_sb = singles.tile([128, KT, H], DT, tag="w2t")
            nc.vector.tensor_copy(w2t_sb, w2t_f)
            w1t_f = singles.tile([128, KT, H], f32, tag="w1tf")
            nc.sync.dma_start(out=w1t_f,
                              in_=w1t_d.rearrange("(kt p) h -> p kt h", p=128))
            w1t_sb = singles.tile([128, KT, H], DT, tag="w1t")
            nc.vector.tensor_copy(w1t_sb, w1t_f)
            hidT_f = singles.tile([128, KT, BPC], f32, tag="hidTf")
            nc.sync.dma_start(out=hidT_f,
                              in_=hidT_d.rearrange("(kt p) b -> p kt b", p=128))
            hidT_sb = singles.tile([128, KT, BPC], DT, tag="hidT")
            nc.vector.tensor_copy(hidT_sb, hidT_f)
            b_sb = singles.tile([128, HT], f32, tag="bvec")
            nc.sync.dma_start(out=b_sb,
                              in_=bvec_d.rearrange("(kt p) -> p kt", p=128))
            vmat_f = singles.tile([128, HT, 36, BPC], f32, tag="vmatf")
            nc.sync.dma_start(out=vmat_f,
                              in_=vmat_d.rearrange("kt p m b -> p kt m b"))
            vmat_sb = singles.tile([128, HT, 36, BPC], DT, tag="vmat")
            nc.vector.tensor_copy(vmat_sb, vmat_f)

            # exp/softmax state in the 36-partition layout (rows 0-3, 32-35)
            expstore = singles.tile([36, L], f32, tag="expstore")
            mstore = singles.tile([36, NJ], f32, tag="mstore")
            runm0 = singles.tile([36, 1], f32, tag="runm0")
            nc.vector.memset(runm0, -1e30)
            runs0 = singles.tile([36, 1], f32, tag="runs0")
            nc.vector.memset(runs0, 0.0)
            state = {"m": runm0, "s": runs0}

            # ---- h1T[h, b] = W1T.T @ hiddenT, + bias -> SBUF f32 ----
            h1b_sb = singles.tile([128, HT, BPC], f32, tag="h1b")

            def emit_h1():
                ps_h1 = psP.tile([128, HT, BPC], f32, tag="pspre")
                for ht in range(HT):
                    for kt in range(KT):
                        nc.tensor.matmul(ps_h1[:, ht, :],
                                         w1t_sb[:, kt, ts(ht, 128)],
                                         hidT_sb[:, kt, :],
                                         start=(kt == 0), stop=(kt == KT - 1))
                for ht in range(HT):
                    nc.scalar.activation(out=h1b_sb[:, ht, :], in_=ps_h1[:, ht, :],
                                         func=ActF.Identity,
                                         bias=b_sb[:, ht:ht + 1], scale=1.0)

            # ---- batched online-softmax update after l-chunk j lands ----
            HB = BPC // 2
            attn36 = singles.tile([36, L], f32, tag="attn36")

            def emit_jupdate(j, ps_sc):
                jsl = ts(j, GT)
                gm = smp.tile([36, 1], f32, tag="gm")
                nc.vector.reduce_max(out=gm, in_=ps_sc,
                                     axis=mybir.AxisListType.X)
                newm = smp.tile([36, 1], f32, tag="newm")
                nc.vector.tensor_max(newm, state["m"], gm)
                nc.vector.tensor_copy(mstore[:, j:j + 1], newm)
                # rescale factor exp(m_old - m_new) for the running sum
                d = smp.tile([36, 1], f32, tag="d")
                nc.vector.tensor_sub(d, state["m"], newm)
                r = smp.tile([36, 1], f32, tag="r")
                nc.scalar.activation(out=r, in_=d, func=ActF.Exp)
                negm = smp.tile([36, 1], f32, tag="negm")
                nc.vector.tensor_scalar_mul(negm, newm, -1.0)
                csum = smp.tile([36, 1], f32, tag="csum")
                nc.scalar.activation(out=expstore[:, jsl], in_=ps_sc,
                                     func=ActF.Exp, bias=negm[:, 0:1],
                                     scale=1.0, accum_out=csum)
                srs = smp.tile([36, 1], f32, tag="srs")
                nc.vector.tensor_mul(srs, state["s"], r)
                news = smp.tile([36, 1], f32, tag="news")
                nc.vector.tensor_add(news, srs, csum)
                state["m"], state["s"] = newm, news

            def emit_final():
                rinv = smp.tile([36, 1], f32, tag="rinv")
                nc.vector.reciprocal(rinv, state["s"])
                # all NJ chunk scales in one shot: exp(m_j - m) / s
                dall = smp.tile([36, NJ], f32, tag="dall")
                nc.vector.tensor_scalar_sub(dall, mstore, state["m"][:, 0:1])
                eall = smp.tile([36, NJ], f32, tag="eall")
                nc.scalar.activation(out=eall, in_=dall, func=ActF.Exp)
                sc_all = smp.tile([36, NJ], f32, tag="sc_all")
                nc.vector.tensor_scalar_mul(sc_all, eall, rinv[:, 0:1])
                for j in range(NJ):
                    if j < 2:
                        nc.vector.tensor_scalar_mul(attn36[:, ts(j, GT)],
                                                    expstore[:, ts(j, GT)],
                                                    sc_all[:, j:j + 1])
                    else:
                        # route half the rescale muls to ACT for parallelism
                        nc.scalar.mul(attn36[:, ts(j, GT)],
                                      expstore[:, ts(j, GT)],
                                      sc_all[:, j:j + 1])
                nc.sync.dma_start(out=attn_d[0:HB, :], in_=attn36[0:HB, :])
                nc.scalar.dma_start(out=attn_d[HB:, :], in_=attn36[32:32 + HB, :])

            # ---- main 4-stage software pipeline, j-major over (j, b) ----
            enc_r = enc_d.rearrange("(g n p) k -> g p n k", g=G, p=128)
            enc_tiles = {}
            encT_tiles = {}
            energy_tiles = {}

            def seq_bj(i):
                return i % BPC, i // BPC      # b, j

            def stage_dma(i):
                b, j = seq_bj(i)
                t = encp.tile([128, GT // 128, H], DT, tag="enc")
                if variant == "nodma":
                    nc.vector.memset(t[:, 0, 0:1], 0.0)
                else:
                    nc.gpsimd.dma_start(out=t, in_=enc_r[b * NJ + j])
                enc_tiles[i] = t

            def stage_transpose(i):
                t = enc_tiles.pop(i)
                st = enctp.tile([128, KT, GT // 128, 128], DT, tag="enct")
                if variant == "notrans":
                    encT_tiles[i] = st
                    return
                # n-outer so the first PE work needs only the first quarter
                # of the group's DMA
                for n in range(GT // 128):
                    pt = psT.tile([128, KT * 128], DT, tag="pst")
                    for kt in range(KT):
                        nc.tensor.matmul(pt[:, ts(kt, 128)],
                                         t[:, n, ts(kt, 128)], ident_r,
                                         is_transpose=True, start=True, stop=True)
                    nc.vector.tensor_copy(st[:, :, n, :], pt)
                encT_tiles[i] = st

            def stage_mm(i):
                b, j = seq_bj(i)
                st = encT_tiles.pop(i)
                energies = []
                for ht in range(HT):
                    ps_pre = psP.tile([128, GT], f32, tag="pspre")
                    for kt in range(KT):
                        nc.tensor.matmul(ps_pre, w2t_sb[:, kt, ts(ht, 128)],
                                         st[:, kt, :, :],
                                         start=(kt == 0), stop=(kt == KT - 1))
                    en = enrgp.tile([128, GT], DT, tag="energy")
                    nc.scalar.activation(out=en, in_=ps_pre, func=ActF.Tanh,
                                         bias=h1b_sb[:, ht, b:b + 1], scale=1.0)
                    energies.append(en)
                energy_tiles[i] = energies

            chunk_psum = {}

            def stage_vdot(i):
                b, j = seq_bj(i)
                energies = energy_tiles.pop(i)
                if variant == "novdot":
                    return
                if b == 0:
                    chunk_psum[j] = psS.tile([36, GT], f32, tag="pssc",
                                             name=f"pssc{j}")
                ps_sc = chunk_psum[j]
                for ht in range(HT):
                    nc.tensor.matmul(ps_sc, vmat_sb[:, ht, :, b], energies[ht],
                                     start=(b == 0 and ht == 0),
                                     stop=(b == BPC - 1 and ht == HT - 1),
                                     skip_group_check=True)
                if b == BPC - 1:
                    emit_jupdate(j, chunk_psum.pop(j))
                    if j == NJ - 1:
                        emit_final()

            for it in range(G + 3):
                if it < G:
                    stage_dma(it)
                if 1 <= it <= G:
                    stage_transpose(it - 1)
                if it == 1:
                    emit_h1()
                if 3 <= it:
                    stage_vdot(it - 3)
                if 2 <= it <= G + 1:
                    stage_mm(it - 2)

    nc.compile()
    return nc


class _Runner:
    """Compile once; jit once; run many times (mirrors run_bass_via_pjrt)."""

    def __init__(self):
        import jax
        import concourse.mybir as mybir
        from concourse.bass2jax import (_bass_exec_p, install_neuronx_cc_hook,
                                        partition_id_tensor)
        from jax.sharding import Mesh, PartitionSpec
        from jax.experimental.shard_map import shard_map

        install_neuronx_cc_hook()
        nc = _build()
        self.nc = nc

        in_names, out_names, out_avals = [], [], []
        for alloc in nc.m.functions[0].allocations:
            if not isinstance(alloc, mybir.MemoryLocationSet):
                continue
            name = alloc.memorylocations[0].name
            if alloc.kind == "ExternalInput":
                in_names.append(name)
            elif alloc.kind == "ExternalOutput":
                out_names.append(name)
                out_avals.append(jax.core.ShapedArray(
                    tuple(alloc.tensor_shape), mybir.dt.np(alloc.dtype)))
        part_name = (nc.partition_id_tensor.name
                     if nc.partition_id_tensor is not None else None)
        if part_name is not None and part_name in in_names:
            in_names.remove(part_name)
        self.in_names, self.out_names, self.out_avals = in_names, out_names, out_avals
        n_params = len(in_names)
        n_outs = len(out_names)
        all_names = in_names + out_names
        if part_name is not None:
            all_names = all_names + [part_name]

        def _body(*args):
            operands = list(args)
            if part_name is not None:
                operands.append(partition_id_tensor())
            return tuple(_bass_exec_p.bind(
                *operands,
                out_avals=tuple(out_avals),
                in_names=tuple(all_names),
                out_names=tuple(out_names),
                lowering_input_output_aliases=(),
                sim_require_finite=True,
                sim_require_nnan=True,
                nc=nc,
            ))

        devices = jax.devices()[:NCORES]
        self.mesh = Mesh(np.asarray(devices), ("core",))
        in_specs = (PartitionSpec("core"),) * (n_params + n_outs)
        out_specs = (PartitionSpec("core"),) * n_outs
        self.jit = jax.jit(
            shard_map(_body, mesh=self.mesh, in_specs=in_specs,
                      out_specs=out_specs, check_rep=False),
            donate_argnums=tuple(range(n_params, n_params + n_outs)),
            keep_unused=True,
        )
        self.zero_outs = [np.zeros((NCORES * a.shape[0], *a.shape[1:]), a.dtype)
                          for a in out_avals]

    def run(self, concat_ins):
        outs = self.jit(*concat_ins, *self.zero_outs)
        return outs


_runner = None


def _get_runner():
    global _runner
    if _runner is None:
        _runner = _Runner()
    return _runner


def prepare_inputs(hidden, encoder_outputs, W, b, v):
    """Host-side shard + layout prep -> concat arrays in runner input order."""
    hidden = np.ascontiguousarray(hidden, dtype=np.float32)
    encoder_outputs = np.ascontiguousarray(encoder_outputs, dtype=np.float32)
    W = np.ascontiguousarray(W, dtype=np.float32)
    b = np.ascontiguousarray(b, dtype=np.float32)
    v = np.ascontiguousarray(v, dtype=np.float32)

    w1t = np.ascontiguousarray(W[:, :H].T)          # [k, h]
    w2t = np.ascontiguousarray(W[:, H:].T)          # [k, h]
    vmat = np.zeros((HT, 128, 36, BPC), np.float32)
    for bb in range(BPC):
        r = 32 * (bb // (BPC // 2)) + bb % (BPC // 2)
        vmat[:, :, r, bb] = v.reshape(HT, 128)
    ident = np.eye(128, dtype=np.float16)

    # per-core shards are contiguous and in core order, so the "concatenated"
    # enc is just a reshape view — avoids a 268 MB host memcpy per call
    concat = {
        "enc": encoder_outputs.reshape(NCORES * T, H),
        "w2t": np.tile(w2t, (NCORES, 1)),
        "w1t": np.tile(w1t, (NCORES, 1)),
        "hidT": np.concatenate(
            [np.ascontiguousarray(hidden[c * BPC:(c + 1) * BPC].T)
             for c in range(NCORES)], axis=0),
        "bvec": np.tile(b, NCORES),
        "vmat": np.tile(vmat, (NCORES, 1, 1, 1)),
        "ident": np.tile(ident, (NCORES, 1)),
    }
    runner = _get_runner()
    return [concat[name] for name in runner.in_names]


def kernel(hidden, encoder_outputs, W, b, v):
    runner = _get_runner()
    concat_ins = prepare_inputs(hidden, encoder_outputs, W, b, v)
    outs = runner.run(concat_ins)
    (iattn,) = [i for i, n in enumerate(runner.out_names) if n == "attn"]
    attn = np.asarray(outs[iattn])          # [NCORES*BPC, L]
    return attn.reshape(B, 1, L)


# revision 37
# speedup vs baseline: 1.0005x; 1.0005x over previous
#!/usr/bin/env python3
"""Bass/Trainium2 kernel for nn_Attention_12747462934680.

Reference computation (B=64, L=2048, H=512):
    x = concat([hidden broadcast over L, encoder_outputs], -1)   # [B, L, 2H]
    energy = tanh(x @ W.T + b)                                   # [B, L, H]
    scores = energy @ v                                          # [B, L]
    attn = softmax(scores, axis=1)[:, None, :]                   # [B, 1, L]

Decomposition:
    pre[b,l,h] = (enc[b,l] @ W2.T)[h] + (hidden[b] @ W1.T)[h] + bias[h]
    with W1 = W[:, :H], W2 = W[:, H:].  The hidden term is per-(b,h), computed
    once; the big matmul is enc @ W2.T.

Sharding: data-parallel over B across 8 cores (8 batches/core).

Per-core device pipeline (SPMD, no collectives), data path in fp16
(~11-bit mantissa — same error class as the PE's tf32-like f32r mode, but
transposes run at 1 cyc/row and weight loads get FWL):
  - h1T[h, b] = W1T.T @ hiddenT + bias  (tiny matmul, ACT adds bias)
  - software-pipelined loop over 32 (l-chunk j, batch b) groups of 512
    tokens, j-major so each l-chunk completes for all 8 b before the next:
      SWDGE DMA enc[512, 512] natural, casting f32 -> fp16
      16x PE transpose [128,128] -> PSUM, DVE copy -> encT (k on partitions)
      preT[h, t] = W2T.T @ encT  (fp16 matmul, fp32 PSUM)
      energy = tanh(preT + h1T[:, b]) on ACT (PSUM -> SBUF, fp16)
      scores: vmat.T @ energy with M=36; vmat is one-hot so batch b lands on
      32-aligned row 32*(b//4)+b%4, and all 8 b of a chunk ACCUMULATE into
      one [36, 512] PSUM tile -> the chunk's scores assemble in PSUM with no
      partition-scatter DMAs (engines cannot shift partitions)
  - once a chunk completes: online-softmax update reads that PSUM directly:
    running max m, exp-chunk into expstore[36, L] (ACT, accum_out gives the
    chunk sum for free), running rescaled sum s = s*exp(m_old-m_new) + csum
  - tail is only the final rescale: attn = expstore * exp(m_j - m)/s with
    vectorized scale math, two output DMAs  (~7 us instead of 14)
"""
import sys
import numpy as np

sys.path.insert(0, "/opt/trn_rl_repo")

B, L, H = 64, 2048, 512
NCORES = 8
BPC = B // NCORES          # batches per core
T = BPC * L                # tokens per core = 16384
GT = 512                   # tokens per group
G = T // GT                # 32 groups
NJ = L // GT               # 4 l-chunks per batch
KT = H // 128              # 4 k-tiles
HT = H // 128              # 4 h-tiles

_compiled = None


def _build(variant="full"):
    from contextlib import ExitStack
    from concourse import bacc, mybir
    import concourse.tile as tile
    from concourse.bass import ts

    f32 = mybir.dt.float32
    fp16 = mybir.dt.float16
    DT = fp16
    ActF = mybir.ActivationFunctionType

    nc = bacc.Bacc("TRN2", target_bir_lowering=False, debug=False,
                   enable_asserts=True, num_devices=NCORES)

    enc_d = nc.dram_tensor("enc", [T, H], f32, kind="ExternalInput").ap()
    w2t_d = nc.dram_tensor("w2t", [H, H], f32, kind="ExternalInput").ap()
    w1t_d = nc.dram_tensor("w1t", [H, H], f32, kind="ExternalInput").ap()
    hidT_d = nc.dram_tensor("hidT", [H, BPC], f32, kind="ExternalInput").ap()
    bvec_d = nc.dram_tensor("bvec", [H], f32, kind="ExternalInput").ap()
    vmat_d = nc.dram_tensor("vmat", [HT, 128, 36, BPC], f32,
                            kind="ExternalInput").ap()
    ident_d = nc.dram_tensor("ident", [128, 128], fp16, kind="ExternalInput").ap()
    attn_d = nc.dram_tensor("attn", [BPC, L], f32, kind="ExternalOutput").ap()

    with tile.TileContext(nc) as tc:
        with ExitStack() as ctx:
            singles = ctx.enter_context(tc.tile_pool(name="singles", bufs=1))
            encp = ctx.enter_context(tc.tile_pool(name="encp", bufs=3))
            enctp = ctx.enter_context(tc.tile_pool(name="enctp", bufs=3))
            enrgp = ctx.enter_context(tc.tile_pool(name="enrgp", bufs=10))
            smp = ctx.enter_context(tc.tile_pool(name="smp", bufs=3))
            psT = ctx.enter_context(tc.tile_pool(name="psT", bufs=3, space="PSUM"))
            psP = ctx.enter_context(tc.tile_pool(name="psP", bufs=3, space="PSUM"))
            psS = ctx.enter_context(tc.tile_pool(name="psS", bufs=2, space="PSUM"))

            # ---- constants / params ----
            # params ride the HWDGE (sync) queue so SWDGE streams enc
            # immediately; DVE casts them to fp16.
            ident_r = singles.tile([128, 128], DT, tag="identr")
            nc.sync.dma_start(out=ident_r, in_=ident_d)
            # h1 params first on the sync ring so the PE can run h1's
            # matmuls during the group-0 enc DMA wait
            w1t_f = singles.tile([128, KT, H], f32, tag="w1tf")
            nc.sync.dma_start(out=w1t_f,
                              in_=w1t_d.rearrange("(kt p) h -> p kt h", p=128))
            w1t_sb = singles.tile([128, KT, H], DT, tag="w1t")
            nc.vector.tensor_copy(w1t_sb, w1t_f)
            hidT_f = singles.tile([128, KT, BPC], f32, tag="hidTf")
            nc.sync.dma_start(out=hidT_f,
                              in_=hidT_d.rearrange("(kt p) b -> p kt b", p=128))
            hidT_sb = singles.tile([128, KT, BPC], DT, tag="hidT")
            nc.vector.tensor_copy(hidT_sb, hidT_f)
            w2t_f = singles.tile([128, KT, H], f32, tag="w2tf")
            nc.sync.dma_start(out=w2t_f,
                              in_=w2t_d.rearrange("(kt p) h -> p kt h", p=128))
            w2t_sb = singles.tile([128, KT, H], DT, tag="w2t")
            nc.vector.tensor_copy(w2t_sb, w2t_f)
            b_sb = singles.tile([128, HT], f32, tag="bvec")
            nc.sync.dma_start(out=b_sb,
                              in_=bvec_d.rearrange("(kt p) -> p kt", p=128))
            vmat_f = singles.tile([128, HT, 36, BPC], f32, tag="vmatf")
            nc.sync.dma_start(out=vmat_f,
                              in_=vmat_d.rearrange("kt p m b -> p kt m b"))
            vmat_sb = singles.tile([128, HT, 36, BPC], DT, tag="vmat")
            nc.vector.tensor_copy(vmat_sb, vmat_f)

            # exp/softmax state in the 36-partition layout (rows 0-3, 32-35)
            expstore = singles.tile([36, L], f32, tag="expstore")
            mstore = singles.tile([36, NJ], f32, tag="mstore")
            runm0 = singles.tile([36, 1], f32, tag="runm0")
            nc.vector.memset(runm0, -1e30)
            runs0 = singles.tile([36, 1], f32, tag="runs0")
            nc.vector.memset(runs0, 0.0)
            state = {"m": runm0, "s": runs0}

            # ---- h1T[h, b] = W1T.T @ hiddenT, + bias -> SBUF f32 ----
            h1b_sb = singles.tile([128, HT, BPC], f32, tag="h1b")

            def emit_h1():
                ps_h1 = psP.tile([128, HT, BPC], f32, tag="pspre")
                for ht in range(HT):
                    for kt in range(KT):
                        nc.tensor.matmul(ps_h1[:, ht, :],
                                         w1t_sb[:, kt, ts(ht, 128)],
                                         hidT_sb[:, kt, :],
                                         start=(kt == 0), stop=(kt == KT - 1))
                for ht in range(HT):
                    nc.scalar.activation(out=h1b_sb[:, ht, :], in_=ps_h1[:, ht, :],
                                         func=ActF.Identity,
                                         bias=b_sb[:, ht:ht + 1], scale=1.0)

            # ---- batched online-softmax update after l-chunk j lands ----
            HB = BPC // 2
            attn36 = singles.tile([36, L], f32, tag="attn36")

            def emit_jupdate(j, ps_sc):
                jsl = ts(j, GT)
                gm = smp.tile([36, 1], f32, tag="gm")
                nc.vector.reduce_max(out=gm, in_=ps_sc,
                                     axis=mybir.AxisListType.X)
                newm = smp.tile([36, 1], f32, tag="newm")
                nc.vector.tensor_max(newm, state["m"], gm)
                nc.vector.tensor_copy(mstore[:, j:j + 1], newm)
                # rescale factor exp(m_old - m_new) for the running sum
                d = smp.tile([36, 1], f32, tag="d")
                nc.vector.tensor_sub(d, state["m"], newm)
                r = smp.tile([36, 1], f32, tag="r")
                nc.scalar.activation(out=r, in_=d, func=ActF.Exp)
                negm = smp.tile([36, 1], f32, tag="negm")
                nc.vector.tensor_scalar_mul(negm, newm, -1.0)
                csum = smp.tile([36, 1], f32, tag="csum")
                nc.scalar.activation(out=expstore[:, jsl], in_=ps_sc,
                                     func=ActF.Exp, bias=negm[:, 0:1],
                                     scale=1.0, accum_out=csum)
                srs = smp.tile([36, 1], f32, tag="srs")
                nc.vector.tensor_mul(srs, state["s"], r)
                news = smp.tile([36, 1], f32, tag="news")
                nc.vector.tensor_add(news, srs, csum)
                state["m"], state["s"] = newm, news

            def emit_final():
                rinv = smp.tile([36, 1], f32, tag="rinv")
                nc.vector.reciprocal(rinv, state["s"])
                # all NJ chunk scales in one shot: exp(m_j - m) / s
                dall = smp.tile([36, NJ], f32, tag="dall")
                nc.vector.tensor_scalar_sub(dall, mstore, state["m"][:, 0:1])
                eall = smp.tile([36, NJ], f32, tag="eall")
                nc.scalar.activation(out=eall, in_=dall, func=ActF.Exp)
                sc_all = smp.tile([36, NJ], f32, tag="sc_all")
                nc.vector.tensor_scalar_mul(sc_all, eall, rinv[:, 0:1])
                for j in range(NJ):
                    if j < 2:
                        nc.vector.tensor_scalar_mul(attn36[:, ts(j, GT)],
                                                    expstore[:, ts(j, GT)],
                                                    sc_all[:, j:j + 1])
                    else:
                        # route half the rescale muls to ACT for parallelism
                        nc.scalar.mul(attn36[:, ts(j, GT)],
                                      expstore[:, ts(j, GT)],
                                      sc_all[:, j:j + 1])
                nc.sync.dma_start(out=attn_d[0:HB, :], in_=attn36[0:HB, :])
                nc.scalar.dma_start(out=attn_d[HB:, :], in_=attn36[32:32 + HB, :])

            # ---- main 4-stage software pipeline, j-major over (j, b) ----
            enc_r = enc_d.rearrange("(g n p) k -> g p n k", g=G, p=128)
            enc_tiles = {}
            encT_tiles = {}
            energy_tiles = {}

            def seq_bj(i):
                return i % BPC, i // BPC      # b, j

            def stage_dma(i):
                b, j = seq_bj(i)
                t = encp.tile([128, GT // 128, H], DT, tag="enc")
                if variant == "nodma":
                    nc.vector.memset(t[:, 0, 0:1], 0.0)
                else:
                    nc.gpsimd.dma_start(out=t, in_=enc_r[b * NJ + j])
                enc_tiles[i] = t

            def stage_transpose(i):
                t = enc_tiles.pop(i)
                st = enctp.tile([128, KT, GT // 128, 128], DT, tag="enct")
                if variant == "notrans":
                    encT_tiles[i] = st
                    return
                # n-outer so the first PE work needs only the first quarter
                # of the group's DMA
                for n in range(GT // 128):
                    pt = psT.tile([128, KT * 128], DT, tag="pst")
                    for kt in range(KT):
                        nc.tensor.matmul(pt[:, ts(kt, 128)],
                                         t[:, n, ts(kt, 128)], ident_r,
                                         is_transpose=True, start=True, stop=True)
                    nc.vector.tensor_copy(st[:, :, n, :], pt)
                encT_tiles[i] = st

            def stage_mm(i):
                b, j = seq_bj(i)
                st = encT_tiles.pop(i)
                energies = []
                for ht in range(HT):
                    ps_pre = psP.tile([128, GT], f32, tag="pspre")
                    for kt in range(KT):
                        nc.tensor.matmul(ps_pre, w2t_sb[:, kt, ts(ht, 128)],
                                         st[:, kt, :, :],
                                         start=(kt == 0), stop=(kt == KT - 1))
                    en = enrgp.tile([128, GT], DT, tag="energy")
                    nc.scalar.activation(out=en, in_=ps_pre, func=ActF.Tanh,
                                         bias=h1b_sb[:, ht, b:b + 1], scale=1.0)
                    energies.append(en)
                energy_tiles[i] = energies

            chunk_psum = {}

            def stage_vdot(i):
                b, j = seq_bj(i)
                energies = energy_tiles.pop(i)
                if variant == "novdot":
                    return
                if b == 0:
                    chunk_psum[j] = psS.tile([36, GT], f32, tag="pssc",
                                             name=f"pssc{j}")
                ps_sc = chunk_psum[j]
                for ht in range(HT):
                    nc.tensor.matmul(ps_sc, vmat_sb[:, ht, :, b], energies[ht],
                                     start=(b == 0 and ht == 0),
                                     stop=(b == BPC - 1 and ht == HT - 1),
                                     skip_group_check=True)
                if b == BPC - 1:
                    emit_jupdate(j, chunk_psum.pop(j))
                    if j == NJ - 1:
                        emit_final()

            for it in range(G + 3):
                if it < G:
                    stage_dma(it)
                if it == 0:
                    emit_h1()
                if 1 <= it <= G:
                    stage_transpose(it - 1)
                if 3 <= it:
                    stage_vdot(it - 3)
                if 2 <= it <= G + 1:
                    stage_mm(it - 2)

    nc.compile()
    return nc


class _Runner:
    """Compile once; jit once; run many times (mirrors run_bass_via_pjrt)."""

    def __init__(self):
        import jax
        import concourse.mybir as mybir
        from concourse.bass2jax import (_bass_exec_p, install_neuronx_cc_hook,
                                        partition_id_tensor)
        from jax.sharding import Mesh, PartitionSpec
        from jax.experimental.shard_map import shard_map

        install_neuronx_cc_hook()
        nc = _build()
        self.nc = nc

        in_names, out_names, out_avals = [], [], []
        for alloc in nc.m.functions[0].allocations:
            if not isinstance(alloc, mybir.MemoryLocationSet):
                continue
            name = alloc.memorylocations[0].name
            if alloc.kind == "ExternalInput":
                in_names.append(name)
            elif alloc.kind == "ExternalOutput":
                out_names.append(name)
                out_avals.append(jax.core.ShapedArray(
                    tuple(alloc.tensor_shape), mybir.dt.np(alloc.dtype)))
        part_name = (nc.partition_id_tensor.name
                     if nc.partition_id_tensor is not None else None)
        if part_name is not None and part_name in in_names:
            in_names.remove(part_name)
        self.in_names, self.out_names, self.out_avals = in_names, out_names, out_avals
        n_params = len(in_names)
        n_outs = len(out_names)
        all_names = in_names + out_names
        if part_name is not None:
            all_names = all_names + [part_name]

        def _body(*args):
            operands = list(args)
            if part_name is not None:
                operands.append(partition_id_tensor())
            return tuple(_bass_exec_p.bind(
                *operands,
                out_avals=tuple(out_avals),
                in_names=tuple(all_names),
                out_names=tuple(out_names),
                lowering_input_output_aliases=(),
                sim_require_finite=True,
                sim_require_nnan=True,
                nc=nc,
            ))

        devices = jax.devices()[:NCORES]
        self.mesh = Mesh(np.asarray(devices), ("core",))
        in_specs = (PartitionSpec("core"),) * (n_params + n_outs)
        out_specs = (PartitionSpec("core"),) * n_outs
        self.jit = jax.jit(
            shard_map(_body, mesh=self.mesh, in_specs=in_specs,
                      out_specs=out_specs, check_rep=False),
            donate_argnums=tuple(range(n_params, n_params + n_outs)),
            keep_unused=True,
        )
        self.zero_outs = [np.zeros((NCORES * a.shape[0], *a.shape[1:]), a.dtype)
                          for a in out_avals]

    def run(self, concat_ins):
        outs = self.jit(*concat_ins, *self.zero_outs)
        return outs


_runner = None


def _get_runner():
    global _runner
    if _runner is None:
        _runner = _Runner()
    return _runner


def prepare_inputs(hidden, encoder_outputs, W, b, v):
    """Host-side shard + layout prep -> concat arrays in runner input order."""
    hidden = np.ascontiguousarray(hidden, dtype=np.float32)
    encoder_outputs = np.ascontiguousarray(encoder_outputs, dtype=np.float32)
    W = np.ascontiguousarray(W, dtype=np.float32)
    b = np.ascontiguousarray(b, dtype=np.float32)
    v = np.ascontiguousarray(v, dtype=np.float32)

    w1t = np.ascontiguousarray(W[:, :H].T)          # [k, h]
    w2t = np.ascontiguousarray(W[:, H:].T)          # [k, h]
    vmat = np.zeros((HT, 128, 36, BPC), np.float32)
    for bb in range(BPC):
        r = 32 * (bb // (BPC // 2)) + bb % (BPC // 2)
        vmat[:, :, r, bb] = v.reshape(HT, 128)
    ident = np.eye(128, dtype=np.float16)

    # per-core shards are contiguous and in core order, so the "concatenated"
    # enc is just a reshape view — avoids a 268 MB host memcpy per call
    concat = {
        "enc": encoder_outputs.reshape(NCORES * T, H),
        "w2t": np.tile(w2t, (NCORES, 1)),
        "w1t": np.tile(w1t, (NCORES, 1)),
        "hidT": np.concatenate(
            [np.ascontiguousarray(hidden[c * BPC:(c + 1) * BPC].T)
             for c in range(NCORES)], axis=0),
        "bvec": np.tile(b, NCORES),
        "vmat": np.tile(vmat, (NCORES, 1, 1, 1)),
        "ident": np.tile(ident, (NCORES, 1)),
    }
    runner = _get_runner()
    return [concat[name] for name in runner.in_names]


def kernel(hidden, encoder_outputs, W, b, v):
    runner = _get_runner()
    concat_ins = prepare_inputs(hidden, encoder_outputs, W, b, v)
    outs = runner.run(concat_ins)
    (iattn,) = [i for i, n in enumerate(runner.out_names) if n == "attn"]
    attn = np.asarray(outs[iattn])          # [NCORES*BPC, L]
    return attn.reshape(B, 1, L)
